# revision 7
# baseline (speedup 1.0000x reference)
"""Trainium2 Bass kernel for a 1-layer LSTM (B=2048, T=512, I=4, H=64) + FC (O=4).

Sharding: data-parallel over batch across 8 NeuronCores (256 examples/core);
the tiny LSTM/FC weights are replicated.

On-core layout ("transposed state"): SBUF partitions carry gate/hidden rows,
the free dimension carries batch.  The 256 local examples form two groups of
128; the two groups are stacked in the partition dimension (group 0 -> rows
0-63, group 1 -> rows 64-127) so ScalarE/VectorE instructions run with all
128 lanes busy and one instruction advances both groups.

Recurrent step t (lockstep over both groups, batch N=128 per group):
  z_g = [h_g (rows 0-63); ones (row 64); x_t^T (rows 65-68)]   # SBUF [69,128]
  8 matmuls (4 gate chunks x 2 groups), K=69, M=64, N=128:
      psA[128, 384] = [i | f | o]   (both groups stacked in partitions)
      psB[128, 128] = g-chunk
  sact = sigmoid(psA); tg = tanh(psB)          # 2 ScalarE instrs
  u = si*tg ; w = sf*c ; c = u + w             # 3 VectorE instrs [128,128]
  tc = tanh(c)                                 # 1 ScalarE instr
  h_g = so_g * tc_g  -> rows 0-63 of the other z buffer (group 1 needs a
      cross-quadrant partition shift, done as two 32-partition VectorE ops)

The input x is pre-transposed on the host to xT[T, I, B_local] so the
per-step x DMA is 4 contiguous rows.  Bias enters through the ones row of z;
the FC bias through the same ones row at the end.
"""

from contextlib import ExitStack

import numpy as np

import concourse.bass as bass
import concourse.tile as tile
from concourse import bacc, mybir
from concourse.bass_utils import run_bass_kernel_spmd

F32 = mybir.dt.float32
BF16 = mybir.dt.bfloat16
AF = mybir.ActivationFunctionType

H, I, O = 64, 4, 4
B, T_FULL = 2048, 512
NCORES = 8
BLOC = B // NCORES          # 256 examples per core
NG = 128                    # batch per group (2 groups per core)
KZ = H + 1 + I              # 69 rows of z: h, ones, x

# bf16 compute (matmuls, activations, cell state) keeps max rel err ~4e-3
# (measured against an f64 oracle) while roughly halving VectorE time.
USE_BF16 = True

# PE DVFS pump: the Tensor engine only reaches its max clock (2.4 GHz) after
# ~3us of *continuous* execution; any idle gap resets it to 1.2 GHz.  The
# recurrence leaves the PE idle most of each step, so real matmuls run at
# half clock.  N_PUMP rank-1 filler matmuls per step (no data deps; scratch
# PSUM output) keep the PE queue non-empty so the clock stays ramped.
N_PUMP = 20
PUMP_N = 256  # moving free-size of each filler matmul (~107ns at 2.4 GHz)

# Run group 1's h-update on the GPSIMD (Pool) engine in parallel with group
# 0's on VectorE, so both z buffers are ready ~220ns sooner.
H1_ON_GPSIMD = True


def build_nc(T=T_FULL, use_bf16=None):
    if use_bf16 is None:
        use_bf16 = USE_BF16
    DT = BF16 if use_bf16 else F32
    nc = bacc.Bacc(
        "TRN2",
        target_bir_lowering=False,
        debug=False,
        enable_asserts=False,
        num_devices=NCORES,
    )

    xT = nc.dram_tensor("xT", [T, I, BLOC], DT, kind="ExternalInput")
    wz = nc.dram_tensor("wz", [KZ, 4, H], DT, kind="ExternalInput")
    wz2 = nc.dram_tensor("wz2", [2 * H, 4, H], DT, kind="ExternalInput")
    wfc = nc.dram_tensor("wfc", [KZ, O], DT, kind="ExternalInput")
    wfc2 = nc.dram_tensor("wfc2", [2 * H, O], DT, kind="ExternalInput")
    out = nc.dram_tensor("out", [2, O, NG], F32, kind="ExternalOutput")

    with tile.TileContext(nc) as tc, ExitStack() as ctx:
        persist = ctx.enter_context(tc.tile_pool(name="persist", bufs=1))
        acts = ctx.enter_context(tc.tile_pool(name="acts", bufs=3))
        temps = ctx.enter_context(tc.tile_pool(name="temps", bufs=3))
        psum = ctx.enter_context(tc.tile_pool(name="psum", bufs=2, space="PSUM"))
        psum1 = ctx.enter_context(tc.tile_pool(name="psum1", bufs=1, space="PSUM"))

        wz_sb = persist.tile([KZ, 4, H], DT, tag="wz")
        nc.sync.dma_start(wz_sb[:], wz[:])
        wz2_sb = persist.tile([2 * H, 4, H], DT, tag="wz2")
        nc.sync.dma_start(wz2_sb[:], wz2[:])
        wfc_sb = persist.tile([KZ, O], DT, tag="wfc")
        nc.sync.dma_start(wfc_sb[:], wfc[:])
        wfc2_sb = persist.tile([2 * H, O], DT, tag="wfc2")
        nc.sync.dma_start(wfc2_sb[:], wfc2[:])

        # Persistent state: cell state (both groups stacked) and the two
        # double-buffered z tiles per group.  Group 0's z is [h; 1; x] (K=69,
        # h in partitions 0-63); group 1's is [1; x; zeros; h] (K=128, h in
        # partitions 64-127, zero rows cost nothing on the PE) so BOTH h
        # updates write the same partitions their operands live in.
        c_st = persist.tile([2 * H, NG], DT, tag="c")
        nc.vector.memset(c_st[:], 0.0)
        zbuf = []
        for j in range(2):
            z = persist.tile([KZ, NG], DT, tag=f"z0{j}")
            nc.vector.memset(z[0:H, :], 0.0)        # h0 = 0
            nc.vector.memset(z[H : H + 1, :], 1.0)  # ones row
            zbuf.append(z)
        zbuf2 = []
        for j in range(2):
            z = persist.tile([2 * H, NG], DT, tag=f"z1{j}")
            nc.vector.memset(z[:], 0.0)             # zeros rows + h0 = 0
            nc.vector.memset(z[0:1, :], 1.0)        # ones row (row 0)
            zbuf2.append(z)

        # Scratch PSUM for the DVFS-pump filler matmuls (results unused).
        if N_PUMP:
            pump_ps = psum1.tile([H, PUMP_N], F32, tag="pump")
            # K=1 stationary/moving slices of the persistent weight tile:
            # read-only, so fillers carry no data dependencies.
            pump_stat = wz2_sb[0:1, 0, :]        # [1, H]
            pump_mov = wz2_sb[0:1, :, :]         # [1, 4*H] -> free 256

        # Schedule rationale (latency-bound loop):
        #   - h0 on VectorE, h1 on GPSIMD in parallel: both z buffers ready
        #     ~220ns sooner than two serial VectorE muls.
        #   - MM queue: z0 block [g,i,f,o] then z2 block [g,i,f,o]; sigmoid
        #     is gated by the tail block's o-chunk, tanh(g) runs earlier on
        #     ScalarE in sigmoid's shadow.
        #   - DVE order u (needs tanh g + sigmoid), w, add.
        for t in range(T):
            zc = [zbuf[t % 2], zbuf2[t % 2]]
            zn = [zbuf[(t + 1) % 2], zbuf2[(t + 1) % 2]]

            # x_t for this step (prefetched ~1 step ahead by the sync queue)
            nc.sync.dma_start(zc[0][H + 1 : KZ, :], xT[t, :, 0:NG])
            nc.sync.dma_start(zc[1][1 : 1 + I, :], xT[t, :, NG : 2 * NG])

            psA = psum.tile([2 * H, 3 * NG], F32, tag="psA")  # [i | f | o]
            psB = psum.tile([2 * H, NG], F32, tag="psB")      # g-chunk
            wzs = [wz_sb, wz2_sb]
            for g in range(2):  # z0 block first (its h-mul lands first)
                gp = slice(g * H, (g + 1) * H)
                nc.tensor.matmul(
                    psB[gp, :], wzs[g][:, 2, :], zc[g][:], start=True, stop=True
                )
                for ci, ch in enumerate((0, 1, 3)):  # i, f, o chunks
                    nc.tensor.matmul(
                        psA[gp, ci * NG : (ci + 1) * NG],
                        wzs[g][:, ch, :],
                        zc[g][:],
                        start=True,
                        stop=True,
                    )
            # DVFS pump: fillers queue behind the real matmuls and keep the
            # PE busy until the next step's z is ready.
            for _ in range(N_PUMP):
                nc.tensor.matmul(
                    pump_ps[:], pump_stat, pump_mov, start=True, stop=True
                )

            tg = acts.tile([2 * H, NG], DT, tag="tg")
            nc.scalar.activation(tg[:], psB[:], AF.Tanh)
            sact = acts.tile([2 * H, 3 * NG], DT, tag="sact")
            nc.scalar.activation(sact[:], psA[:], AF.Sigmoid)

            si = sact[:, 0:NG]
            sf = sact[:, NG : 2 * NG]
            so = sact[:, 2 * NG : 3 * NG]

            u = temps.tile([2 * H, NG], DT, tag="u")
            nc.vector.tensor_mul(u[:], si, tg[:])
            w = temps.tile([2 * H, NG], DT, tag="w")
            nc.vector.tensor_mul(w[:], sf, c_st[:])
            nc.vector.tensor_add(c_st[:], u[:], w[:])

            tcs = acts.tile([2 * H, NG], DT, tag="tc")
            nc.scalar.activation(tcs[:], c_st[:], AF.Tanh)

            # h updates: each group writes the partitions it already lives in.
            nc.vector.tensor_mul(zn[0][0:H, :], so[0:H, :], tcs[0:H, :])
            if H1_ON_GPSIMD:
                nc.gpsimd.tensor_mul(zn[1][H : 2 * H, :], so[H:, :], tcs[H:, :])
            else:
                nc.vector.tensor_mul(zn[1][H : 2 * H, :], so[H:, :], tcs[H:, :])

        # Final FC (bias added on the host during the gather).
        zf0, zf1 = zbuf[T % 2], zbuf2[T % 2]
        fc_ps0 = psum1.tile([O, NG], F32, tag="fc0")
        nc.tensor.matmul(fc_ps0[:], wfc_sb[:], zf0[:], start=True, stop=True)
        fc_ps1 = psum1.tile([O, NG], F32, tag="fc1")
        nc.tensor.matmul(fc_ps1[:], wfc2_sb[:], zf1[:], start=True, stop=True)
        for g, fc_ps in enumerate((fc_ps0, fc_ps1)):
            fc_sb = temps.tile([O, NG], F32, tag="fcsb")
            nc.vector.tensor_copy(fc_sb[:], fc_ps[:])
            nc.sync.dma_start(out[g], fc_sb[:])

    nc.compile()
    return nc


def prep_weights(W_ih, W_hh, b_ih, b_hh, W_fc, b_fc):
    bsum = (b_ih + b_hh).astype(np.float32)
    # group 0 z rows: [h (64); ones (1); x (4)] -> [W_hh^T; b; W_ih^T]
    wz = np.empty((KZ, 4, H), np.float32)
    # group 1 z rows: [ones (1); x (4); zeros (59); h (64)]
    wz2 = np.zeros((2 * H, 4, H), np.float32)
    for ch in range(4):
        r = slice(ch * H, (ch + 1) * H)
        wz[0:H, ch, :] = W_hh[r].T
        wz[H, ch, :] = bsum[r]
        wz[H + 1 :, ch, :] = W_ih[r].T
        wz2[0, ch, :] = bsum[r]
        wz2[1 : 1 + I, ch, :] = W_ih[r].T
        wz2[H:, ch, :] = W_hh[r].T
    wfc = np.zeros((KZ, O), np.float32)
    wfc[0:H] = W_fc.T
    wfc2 = np.zeros((2 * H, O), np.float32)
    wfc2[H:] = W_fc.T
    return wz, wz2, wfc, wfc2


def make_in_maps(x, W_ih, W_hh, b_ih, b_hh, W_fc, b_fc, T=T_FULL, use_bf16=None):
    import ml_dtypes

    if use_bf16 is None:
        use_bf16 = USE_BF16
    npdt = ml_dtypes.bfloat16 if use_bf16 else np.float32
    wz, wz2, wfc, wfc2 = prep_weights(W_ih, W_hh, b_ih, b_hh, W_fc, b_fc)
    wz, wz2, wfc, wfc2 = (a.astype(npdt) for a in (wz, wz2, wfc, wfc2))
    in_maps = []
    for core in range(NCORES):
        xc = x[core * BLOC : (core + 1) * BLOC, :T, :]  # [BLOC, T, I]
        xTc = np.ascontiguousarray(xc.transpose(1, 2, 0)).astype(npdt)
        in_maps.append({"xT": xTc, "wz": wz, "wz2": wz2, "wfc": wfc, "wfc2": wfc2})
    return in_maps


_CACHED_NC = None


def kernel(x, W_ih, W_hh, b_ih, b_hh, W_fc, b_fc):
    global _CACHED_NC
    x = np.asarray(x, np.float32)
    args = [np.asarray(a, np.float32) for a in (W_ih, W_hh, b_ih, b_hh, W_fc, b_fc)]
    if _CACHED_NC is None:
        _CACHED_NC = build_nc()
    nc = _CACHED_NC
    in_maps = make_in_maps(x, *args)
    res = run_bass_kernel_spmd(nc, in_maps, core_ids=list(range(NCORES)))
    b_fc = args[5]
    full = np.empty((1, B, O), np.float32)
    for core in range(NCORES):
        oc = res.results[core]["out"]  # [2, O, NG]
        for g in range(2):
            lo = core * BLOC + g * NG
            full[0, lo : lo + NG, :] = oc[g].T + b_fc
    return full



# revision 12
# speedup vs baseline: 1.5785x; 1.5785x over previous
"""Trainium2 Bass kernel for a 1-layer LSTM (B=2048, T=512, I=4, H=64) + FC (O=4).

Sharding: data-parallel over batch across 8 NeuronCores (256 examples/core);
the tiny LSTM/FC weights are replicated.

On-core layout: SBUF partitions carry hidden/gate rows, the free dimension
carries batch.  The 256 local examples form two groups of 128; the groups are
stacked in the partition dimension (group 0 -> rows 0-63, group 1 -> rows
64-127) so ScalarE/VectorE instructions run with all 128 lanes busy.

The recurrent state is a single tile hbuf[128, 128] (both groups' h stacked).
Per step the gate pre-activations are built by PSUM accumulation of two
matmuls per gate chunk:
  mm_x (start=True):  stat Wx2[10, 128]  x  ox2[10, 128]   (bias+x part)
      ox2 rows: [1; x_g0 (4); 1; x_g1 (4)]; Wx2 block-maps each group's
      rows onto its 64 output columns.  These run OFF the critical path
      (the x DMA lands ~2 steps ahead).
  mm_h (stop=True):   stat Wh2[128, 128] x  hbuf[128, 128]  (recurrent part)
      Wh2 = blockdiag(W_hh_chunk^T, W_hh_chunk^T) so ONE matmul covers both
      groups; only 4 h-matmuls gate the step.
Then on ScalarE: tanh(g), sigmoid(i|f) (gates the c-update), sigmoid(o)
(only needed after tanh(c), runs in the ScalarE shadow), tanh(c); on
VectorE: u = si*tg, w = sf*c, c = u + w, and ONE h-update
h = so * tanh(c) -> hbuf (both groups at once).
"""

from contextlib import ExitStack

import numpy as np

import concourse.bass as bass
import concourse.tile as tile
from concourse import bacc, mybir
from concourse.bass_utils import run_bass_kernel_spmd

F32 = mybir.dt.float32
BF16 = mybir.dt.bfloat16
AF = mybir.ActivationFunctionType

H, I, O = 64, 4, 4
B, T_FULL = 2048, 512
NCORES = 8
BLOC = B // NCORES          # 256 examples per core
NG = 128                    # batch per group (2 groups per core)
KX = 2 * (1 + I)            # 10 rows of ox2: [1; x_g0; 1; x_g1]

# bf16 compute (matmuls, activations, cell state) keeps max rel err ~4e-3
USE_BF16 = True

# Chunk order: g first (its tanh runs earliest on ScalarE), then i, f
# (gate sigmoid(i|f), which the c-update waits on), o last.
# psA columns: [i | f | o]; psB: g.
CHUNKS = (2, 0, 1, 3)  # weight-row chunk ids in PE issue order g,i,f,o
PSA_COL = {0: 0, 1: 1, 3: 2}  # chunk id -> psA column block

# Stationary width for the recurrent matmuls: 128 = one block-diagonal
# matmul per chunk (both groups); 64 = two per chunk (one per group,
# cheaper LDWEIGHTS per matmul if the wide load bounds the wave rate).
STAT_M = 128


def build_nc(T=T_FULL, use_bf16=None):
    if use_bf16 is None:
        use_bf16 = USE_BF16
    DT = BF16 if use_bf16 else F32
    nc = bacc.Bacc(
        "TRN2",
        target_bir_lowering=False,
        debug=False,
        enable_asserts=False,
        num_devices=NCORES,
    )

    xT2 = nc.dram_tensor("xT2", [T, KX, NG], DT, kind="ExternalInput")
    wh2 = nc.dram_tensor("wh2", [2 * H, 4, 2 * H], DT, kind="ExternalInput")
    wx2 = nc.dram_tensor("wx2", [KX, 4, 2 * H], DT, kind="ExternalInput")
    wfc2 = nc.dram_tensor("wfc2", [2 * H, 2 * O], DT, kind="ExternalInput")
    out = nc.dram_tensor("out", [2 * O, NG], F32, kind="ExternalOutput")

    with tile.TileContext(nc) as tc, ExitStack() as ctx:
        persist = ctx.enter_context(tc.tile_pool(name="persist", bufs=1))
        acts = ctx.enter_context(tc.tile_pool(name="acts", bufs=3))
        temps = ctx.enter_context(tc.tile_pool(name="temps", bufs=3))
        psum = ctx.enter_context(tc.tile_pool(name="psum", bufs=2, space="PSUM"))
        psum1 = ctx.enter_context(tc.tile_pool(name="psum1", bufs=1, space="PSUM"))

        wh2_sb = persist.tile([2 * H, 4, 2 * H], DT, tag="wh2")
        nc.sync.dma_start(wh2_sb[:], wh2[:])
        wx2_sb = persist.tile([KX, 4, 2 * H], DT, tag="wx2")
        nc.sync.dma_start(wx2_sb[:], wx2[:])
        wfc2_sb = persist.tile([2 * H, 2 * O], DT, tag="wfc2")
        nc.sync.dma_start(wfc2_sb[:], wfc2[:])

        # Persistent state: cell state and the double-buffered hidden state.
        c_st = persist.tile([2 * H, NG], DT, tag="c")
        nc.vector.memset(c_st[:], 0.0)
        hbuf = []
        for j in range(2):
            hb = persist.tile([2 * H, NG], DT, tag=f"h{j}")
            nc.vector.memset(hb[:], 0.0)
            hbuf.append(hb)
        # Double-buffered [1; x] tiles (ones rows come in with the DMA).
        oxb = []
        for j in range(2):
            ox_t = persist.tile([KX, NG], DT, tag=f"ox{j}")
            oxb.append(ox_t)

        for t in range(T):
            hc = hbuf[t % 2]
            hn = hbuf[(t + 1) % 2]
            ox = oxb[t % 2]

            nc.sync.dma_start(ox[:], xT2[t])

            psA = psum.tile([2 * H, 3 * NG], F32, tag="psA")  # [i | f | o]
            psB = psum.tile([2 * H, NG], F32, tag="psB")      # g-chunk

            def gate_out(ch):
                if ch == 2:
                    return psB[:, :]
                ci = PSA_COL[ch]
                return psA[:, ci * NG : (ci + 1) * NG]

            # Per chunk: x/bias part (no h dependency; the PE's lookahead
            # window executes it while the h-matmul ahead of it waits), then
            # the recurrent part closing the accumulation group.  Pairs are
            # adjacent so only one accumulation group is open per region.
            for ch in CHUNKS:
                nc.tensor.matmul(
                    gate_out(ch), wx2_sb[:, ch, :], ox[:], start=True, stop=False
                )
                if STAT_M == 128:
                    nc.tensor.matmul(
                        gate_out(ch), wh2_sb[:, ch, :], hc[:], start=False, stop=True
                    )
                else:
                    for g in range(2):
                        nc.tensor.matmul(
                            gate_out(ch)[g * H : (g + 1) * H, :],
                            wh2_sb[:, ch, g * H : (g + 1) * H],
                            hc[:],
                            start=False,
                            stop=True,
                        )

            tg = acts.tile([2 * H, NG], DT, tag="tg")
            nc.scalar.activation(tg[:], psB[:], AF.Tanh)
            sif = acts.tile([2 * H, 2 * NG], DT, tag="sif")
            nc.scalar.activation(sif[:], psA[:, 0 : 2 * NG], AF.Sigmoid)
            so = acts.tile([2 * H, NG], DT, tag="so")
            nc.scalar.activation(so[:], psA[:, 2 * NG : 3 * NG], AF.Sigmoid)

            si = sif[:, 0:NG]
            sf = sif[:, NG : 2 * NG]

            u = temps.tile([2 * H, NG], DT, tag="u")
            nc.vector.tensor_mul(u[:], si, tg[:])
            w = temps.tile([2 * H, NG], DT, tag="w")
            nc.vector.tensor_mul(w[:], sf, c_st[:])
            nc.vector.tensor_add(c_st[:], u[:], w[:])

            tcs = acts.tile([2 * H, NG], DT, tag="tc")
            nc.scalar.activation(tcs[:], c_st[:], AF.Tanh)

            nc.vector.tensor_mul(hn[:], so[:], tcs[:])

        # Final FC: one matmul, both groups ([O g0 | O g1] output rows).
        hf = hbuf[T % 2]
        fc_ps = psum1.tile([2 * O, NG], F32, tag="fc")
        nc.tensor.matmul(fc_ps[:], wfc2_sb[:], hf[:], start=True, stop=True)
        fc_sb = temps.tile([2 * O, NG], F32, tag="fcsb")
        nc.vector.tensor_copy(fc_sb[:], fc_ps[:])
        nc.sync.dma_start(out[:], fc_sb[:])

    nc.compile()
    return nc


def prep_weights(W_ih, W_hh, b_ih, b_hh, W_fc, b_fc):
    bsum = (b_ih + b_hh).astype(np.float32)
    wh2 = np.zeros((2 * H, 4, 2 * H), np.float32)
    wx2 = np.zeros((KX, 4, 2 * H), np.float32)
    for ch in range(4):
        r = slice(ch * H, (ch + 1) * H)
        wh2[0:H, ch, 0:H] = W_hh[r].T
        wh2[H:, ch, H:] = W_hh[r].T
        wx2[0, ch, 0:H] = bsum[r]
        wx2[1 : 1 + I, ch, 0:H] = W_ih[r].T
        wx2[1 + I, ch, H:] = bsum[r]
        wx2[2 + I :, ch, H:] = W_ih[r].T
    wfc2 = np.zeros((2 * H, 2 * O), np.float32)
    wfc2[0:H, 0:O] = W_fc.T
    wfc2[H:, O:] = W_fc.T
    return wh2, wx2, wfc2


def make_in_maps(x, W_ih, W_hh, b_ih, b_hh, W_fc, b_fc, T=T_FULL, use_bf16=None):
    import ml_dtypes

    if use_bf16 is None:
        use_bf16 = USE_BF16
    npdt = ml_dtypes.bfloat16 if use_bf16 else np.float32
    wh2, wx2, wfc2 = prep_weights(W_ih, W_hh, b_ih, b_hh, W_fc, b_fc)
    wh2, wx2, wfc2 = (a.astype(npdt) for a in (wh2, wx2, wfc2))
    in_maps = []
    for core in range(NCORES):
        xc = x[core * BLOC : (core + 1) * BLOC, :T, :]  # [BLOC, T, I]
        # xT2[t] = [1; x_g0^T; 1; x_g1^T]  -> [T, 10, NG]
        xT = np.ascontiguousarray(xc.transpose(1, 2, 0))  # [T, I, BLOC]
        xT2 = np.empty((T, KX, NG), np.float32)
        xT2[:, 0, :] = 1.0
        xT2[:, 1 : 1 + I, :] = xT[:, :, 0:NG]
        xT2[:, 1 + I, :] = 1.0
        xT2[:, 2 + I :, :] = xT[:, :, NG : 2 * NG]
        in_maps.append(
            {"xT2": xT2.astype(npdt), "wh2": wh2, "wx2": wx2, "wfc2": wfc2}
        )
    return in_maps


_CACHED_NC = None


def kernel(x, W_ih, W_hh, b_ih, b_hh, W_fc, b_fc):
    global _CACHED_NC
    x = np.asarray(x, np.float32)
    args = [np.asarray(a, np.float32) for a in (W_ih, W_hh, b_ih, b_hh, W_fc, b_fc)]
    if _CACHED_NC is None:
        _CACHED_NC = build_nc()
    nc = _CACHED_NC
    in_maps = make_in_maps(x, *args)
    res = run_bass_kernel_spmd(nc, in_maps, core_ids=list(range(NCORES)))
    b_fc = args[5]
    full = np.empty((1, B, O), np.float32)
    for core in range(NCORES):
        oc = res.results[core]["out"]  # [2*O, NG]
        for g in range(2):
            lo = core * BLOC + g * NG
            full[0, lo : lo + NG, :] = oc[g * O : (g + 1) * O].T + b_fc
    return full


# revision 14
# speedup vs baseline: 1.9732x; 1.2501x over previous
"""Trainium2 Bass kernel for a 1-layer LSTM (B=2048, T=512, I=4, H=64) + FC (O=4).

Sharding: data-parallel over batch across 8 NeuronCores (256 examples/core);
the tiny LSTM/FC weights are replicated.

On-core layout: SBUF partitions carry hidden/gate rows, the free dimension
carries batch.  The 256 local examples form two groups of 128; the groups are
stacked in the partition dimension (group 0 -> rows 0-63, group 1 -> rows
64-127) so ScalarE/VectorE instructions run with all 128 lanes busy.

The recurrent state is a single tile hbuf[128, 128] (both groups' h stacked).
Per step the gate pre-activations are built by PSUM accumulation of two
matmuls per gate chunk:
  mm_x (start=True):  stat Wx2[10, 128]  x  ox2[10, 128]   (bias+x part)
      ox2 rows: [1; x_g0 (4); 1; x_g1 (4)]; Wx2 block-maps each group's
      rows onto its 64 output columns.  These run OFF the critical path
      (the x DMA lands ~2 steps ahead).
  mm_h (stop=True):   stat Wh2[128, 128] x  hbuf[128, 128]  (recurrent part)
      Wh2 = blockdiag(W_hh_chunk^T, W_hh_chunk^T) so ONE matmul covers both
      groups; only 4 h-matmuls gate the step.
Then on ScalarE: tanh(g), sigmoid(i|f) (gates the c-update), sigmoid(o)
(only needed after tanh(c), runs in the ScalarE shadow), tanh(c); on
VectorE: u = si*tg, w = sf*c, c = u + w, and ONE h-update
h = so * tanh(c) -> hbuf (both groups at once).
"""

from contextlib import ExitStack

import numpy as np

import concourse.bass as bass
import concourse.tile as tile
from concourse import bacc, mybir
from concourse.bass_utils import run_bass_kernel_spmd

F32 = mybir.dt.float32
BF16 = mybir.dt.bfloat16
AF = mybir.ActivationFunctionType

H, I, O = 64, 4, 4
B, T_FULL = 2048, 512
NCORES = 8
BLOC = B // NCORES          # 256 examples per core
NG = 128                    # batch per group (2 groups per core)
KX = 2 * (1 + I)            # 10 rows of ox2: [1; x_g0; 1; x_g1]

# bf16 compute (matmuls, activations, cell state) keeps max rel err ~4e-3
USE_BF16 = True

# Chunk order: g first (its tanh runs earliest on ScalarE), then i, f
# (gate sigmoid(i|f), which the c-update waits on), o last.
# psA columns: [i | f | o]; psB: g.
CHUNKS = (2, 0, 1, 3)  # weight-row chunk ids in PE issue order g,i,f,o
PSA_COL = {0: 0, 1: 1, 3: 2}  # chunk id -> psA column block

# Stationary width for the recurrent matmuls: 128 = one block-diagonal
# matmul per chunk (both groups); 64 = two per chunk (one per group,
# cheaper LDWEIGHTS per matmul if the wide load bounds the wave rate).
STAT_M = 128


def build_nc(T=T_FULL, use_bf16=None):
    if use_bf16 is None:
        use_bf16 = USE_BF16
    DT = BF16 if use_bf16 else F32
    nc = bacc.Bacc(
        "TRN2",
        target_bir_lowering=False,
        debug=False,
        enable_asserts=False,
        num_devices=NCORES,
    )

    xT2 = nc.dram_tensor("xT2", [T, KX, NG], DT, kind="ExternalInput")
    wh2 = nc.dram_tensor("wh2", [2 * H, 4, 2 * H], DT, kind="ExternalInput")
    wx2 = nc.dram_tensor("wx2", [KX, 4, 2 * H], DT, kind="ExternalInput")
    wfc2 = nc.dram_tensor("wfc2", [2 * H, 2 * O], DT, kind="ExternalInput")
    out = nc.dram_tensor("out", [2 * O, NG], F32, kind="ExternalOutput")

    with tile.TileContext(nc) as tc, ExitStack() as ctx:
        persist = ctx.enter_context(tc.tile_pool(name="persist", bufs=1))
        acts = ctx.enter_context(tc.tile_pool(name="acts", bufs=3))
        temps = ctx.enter_context(tc.tile_pool(name="temps", bufs=3))
        psum = ctx.enter_context(tc.tile_pool(name="psum", bufs=1, space="PSUM"))
        psum1 = ctx.enter_context(tc.tile_pool(name="psum1", bufs=1, space="PSUM"))

        wh2_sb = persist.tile([2 * H, 4, 2 * H], DT, tag="wh2")
        nc.sync.dma_start(wh2_sb[:], wh2[:])
        wx2_sb = persist.tile([KX, 4, 2 * H], DT, tag="wx2")
        nc.sync.dma_start(wx2_sb[:], wx2[:])
        wfc2_sb = persist.tile([2 * H, 2 * O], DT, tag="wfc2")
        nc.sync.dma_start(wfc2_sb[:], wfc2[:])

        # Persistent state: cell state and the double-buffered hidden state.
        c_st = persist.tile([2 * H, NG], DT, tag="c")
        nc.vector.memset(c_st[:], 0.0)
        hbuf = []
        for j in range(2):
            hb = persist.tile([2 * H, NG], DT, tag=f"h{j}")
            nc.vector.memset(hb[:], 0.0)
            hbuf.append(hb)
        # Double-buffered [1; x] tiles (ones rows come in with the DMA).
        oxb = []
        for j in range(2):
            ox_t = persist.tile([KX, NG], DT, tag=f"ox{j}")
            oxb.append(ox_t)

        for t in range(T):
            hc = hbuf[t % 2]
            hn = hbuf[(t + 1) % 2]
            ox = oxb[t % 2]

            nc.sync.dma_start(ox[:], xT2[t])

            # PSUM: `start=True` clears the accumulate (has_written) bits of
            # its whole BANK, so each gate chunk gets a private 2 KB bank:
            # two 2-bank tiles with the second region offset by 512 fp32.
            BK = 512  # fp32 elements per PSUM bank (per partition)
            psIF = psum.tile([2 * H, 2 * BK], F32, tag="psIF")  # i @0, f @512
            psGO = psum.tile([2 * H, 2 * BK], F32, tag="psGO")  # g @0, o @512
            regions = {
                0: psIF[:, 0:NG],            # i
                1: psIF[:, BK : BK + NG],    # f
                2: psGO[:, 0:NG],            # g
                3: psGO[:, BK : BK + NG],    # o
            }

            # x/bias parts: one independent accumulation group per bank; all
            # pre-run in the PE's idle window while the h-matmuls wait for h.
            for ch in CHUNKS:
                nc.tensor.matmul(
                    regions[ch], wx2_sb[:, ch, :], ox[:], start=True, stop=False
                )
            # recurrent parts: the 4-matmul wave gating the step.
            for ch in CHUNKS:
                if STAT_M == 128:
                    nc.tensor.matmul(
                        regions[ch], wh2_sb[:, ch, :], hc[:], start=False, stop=True
                    )
                else:
                    for g in range(2):
                        nc.tensor.matmul(
                            regions[ch][g * H : (g + 1) * H, :],
                            wh2_sb[:, ch, g * H : (g + 1) * H],
                            hc[:],
                            start=False,
                            stop=True,
                        )

            tg = acts.tile([2 * H, NG], DT, tag="tg")
            nc.scalar.activation(tg[:], regions[2], AF.Tanh)
            sif = acts.tile([2 * H, 2 * NG], DT, tag="sif")
            nc.scalar.activation(
                sif[:],
                psIF[:].rearrange("p (b n) -> p b n", b=2)[:, :, 0:NG],
                AF.Sigmoid,
            )
            so = acts.tile([2 * H, NG], DT, tag="so")
            nc.scalar.activation(so[:], regions[3], AF.Sigmoid)

            si = sif[:, 0:NG]
            sf = sif[:, NG : 2 * NG]

            u = temps.tile([2 * H, NG], DT, tag="u")
            nc.vector.tensor_mul(u[:], si, tg[:])
            w = temps.tile([2 * H, NG], DT, tag="w")
            nc.vector.tensor_mul(w[:], sf, c_st[:])
            nc.vector.tensor_add(c_st[:], u[:], w[:])

            tcs = acts.tile([2 * H, NG], DT, tag="tc")
            nc.scalar.activation(tcs[:], c_st[:], AF.Tanh)

            nc.vector.tensor_mul(hn[:], so[:], tcs[:])

        # Final FC: one matmul, both groups ([O g0 | O g1] output rows).
        hf = hbuf[T % 2]
        fc_ps = psum1.tile([2 * O, NG], F32, tag="fc")
        nc.tensor.matmul(fc_ps[:], wfc2_sb[:], hf[:], start=True, stop=True)
        fc_sb = temps.tile([2 * O, NG], F32, tag="fcsb")
        nc.vector.tensor_copy(fc_sb[:], fc_ps[:])
        nc.sync.dma_start(out[:], fc_sb[:])

    nc.compile()
    return nc


def prep_weights(W_ih, W_hh, b_ih, b_hh, W_fc, b_fc):
    bsum = (b_ih + b_hh).astype(np.float32)
    wh2 = np.zeros((2 * H, 4, 2 * H), np.float32)
    wx2 = np.zeros((KX, 4, 2 * H), np.float32)
    for ch in range(4):
        r = slice(ch * H, (ch + 1) * H)
        wh2[0:H, ch, 0:H] = W_hh[r].T
        wh2[H:, ch, H:] = W_hh[r].T
        wx2[0, ch, 0:H] = bsum[r]
        wx2[1 : 1 + I, ch, 0:H] = W_ih[r].T
        wx2[1 + I, ch, H:] = bsum[r]
        wx2[2 + I :, ch, H:] = W_ih[r].T
    wfc2 = np.zeros((2 * H, 2 * O), np.float32)
    wfc2[0:H, 0:O] = W_fc.T
    wfc2[H:, O:] = W_fc.T
    return wh2, wx2, wfc2


def make_in_maps(x, W_ih, W_hh, b_ih, b_hh, W_fc, b_fc, T=T_FULL, use_bf16=None):
    import ml_dtypes

    if use_bf16 is None:
        use_bf16 = USE_BF16
    npdt = ml_dtypes.bfloat16 if use_bf16 else np.float32
    wh2, wx2, wfc2 = prep_weights(W_ih, W_hh, b_ih, b_hh, W_fc, b_fc)
    wh2, wx2, wfc2 = (a.astype(npdt) for a in (wh2, wx2, wfc2))
    in_maps = []
    for core in range(NCORES):
        xc = x[core * BLOC : (core + 1) * BLOC, :T, :]  # [BLOC, T, I]
        # xT2[t] = [1; x_g0^T; 1; x_g1^T]  -> [T, 10, NG]
        xT = np.ascontiguousarray(xc.transpose(1, 2, 0))  # [T, I, BLOC]
        xT2 = np.empty((T, KX, NG), np.float32)
        xT2[:, 0, :] = 1.0
        xT2[:, 1 : 1 + I, :] = xT[:, :, 0:NG]
        xT2[:, 1 + I, :] = 1.0
        xT2[:, 2 + I :, :] = xT[:, :, NG : 2 * NG]
        in_maps.append(
            {"xT2": xT2.astype(npdt), "wh2": wh2, "wx2": wx2, "wfc2": wfc2}
        )
    return in_maps


_CACHED_NC = None


def kernel(x, W_ih, W_hh, b_ih, b_hh, W_fc, b_fc):
    global _CACHED_NC
    x = np.asarray(x, np.float32)
    args = [np.asarray(a, np.float32) for a in (W_ih, W_hh, b_ih, b_hh, W_fc, b_fc)]
    if _CACHED_NC is None:
        _CACHED_NC = build_nc()
    nc = _CACHED_NC
    in_maps = make_in_maps(x, *args)
    res = run_bass_kernel_spmd(nc, in_maps, core_ids=list(range(NCORES)))
    b_fc = args[5]
    full = np.empty((1, B, O), np.float32)
    for core in range(NCORES):
        oc = res.results[core]["out"]  # [2*O, NG]
        for g in range(2):
            lo = core * BLOC + g * NG
            full[0, lo : lo + NG, :] = oc[g * O : (g + 1) * O].T + b_fc
    return full


# revision 15
# speedup vs baseline: 2.0578x; 1.0429x over previous
"""Trainium2 Bass kernel for a 1-layer LSTM (B=2048, T=512, I=4, H=64) + FC (O=4).

Sharding: data-parallel over batch across 8 NeuronCores (256 examples/core);
the tiny LSTM/FC weights are replicated.

On-core layout: SBUF partitions carry hidden/gate rows, the free dimension
carries batch.  The 256 local examples form two groups of 128; the groups are
stacked in the partition dimension (group 0 -> rows 0-63, group 1 -> rows
64-127) so ScalarE/VectorE instructions run with all 128 lanes busy.

The recurrent state is a single tile hbuf[128, 128] (both groups' h stacked).
Per step the gate pre-activations are built by PSUM accumulation of two
matmuls per gate chunk:
  mm_x (start=True):  stat Wx2[10, 128]  x  ox2[10, 128]   (bias+x part)
      ox2 rows: [1; x_g0 (4); 1; x_g1 (4)]; Wx2 block-maps each group's
      rows onto its 64 output columns.  These run OFF the critical path
      (the x DMA lands ~2 steps ahead).
  mm_h (stop=True):   stat Wh2[128, 128] x  hbuf[128, 128]  (recurrent part)
      Wh2 = blockdiag(W_hh_chunk^T, W_hh_chunk^T) so ONE matmul covers both
      groups; only 4 h-matmuls gate the step.
Then on ScalarE: tanh(g), sigmoid(i|f) (gates the c-update), sigmoid(o)
(only needed after tanh(c), runs in the ScalarE shadow), tanh(c); on
VectorE: u = si*tg, w = sf*c, c = u + w, and ONE h-update
h = so * tanh(c) -> hbuf (both groups at once).
"""

from contextlib import ExitStack

import numpy as np

import concourse.bass as bass
import concourse.tile as tile
from concourse import bacc, mybir
from concourse.bass_utils import run_bass_kernel_spmd

F32 = mybir.dt.float32
BF16 = mybir.dt.bfloat16
AF = mybir.ActivationFunctionType

H, I, O = 64, 4, 4
B, T_FULL = 2048, 512
NCORES = 8
BLOC = B // NCORES          # 256 examples per core
NG = 128                    # batch per group (2 groups per core)
KX = 2 * (1 + I)            # 10 rows of ox2: [1; x_g0; 1; x_g1]

# bf16 compute (matmuls, activations, cell state) keeps max rel err ~4e-3
USE_BF16 = True

# PE issue order of the gate chunks (ids: 0=i, 1=f, 2=g, 3=o): f and i
# first — sigmoid(i|f) gates the c-update and issues right after the 2nd
# h-matmul; tanh(g) is ScalarE-shadow-bound behind the sigmoid anyway, so
# g third costs nothing; o (only needed after tanh(c)) last.
CHUNKS = (1, 0, 2, 3)

# Stationary width for the recurrent matmuls: 128 = one block-diagonal
# matmul per chunk (both groups); 64 = two per chunk (one per group,
# cheaper LDWEIGHTS per matmul if the wide load bounds the wave rate).
STAT_M = 128


def build_nc(T=T_FULL, use_bf16=None):
    if use_bf16 is None:
        use_bf16 = USE_BF16
    DT = BF16 if use_bf16 else F32
    nc = bacc.Bacc(
        "TRN2",
        target_bir_lowering=False,
        debug=False,
        enable_asserts=False,
        num_devices=NCORES,
    )

    xT2 = nc.dram_tensor("xT2", [T, KX, NG], DT, kind="ExternalInput")
    wh2 = nc.dram_tensor("wh2", [2 * H, 4, 2 * H], DT, kind="ExternalInput")
    wx2 = nc.dram_tensor("wx2", [KX, 4, 2 * H], DT, kind="ExternalInput")
    wfc2 = nc.dram_tensor("wfc2", [2 * H, 2 * O], DT, kind="ExternalInput")
    out = nc.dram_tensor("out", [2 * O, NG], F32, kind="ExternalOutput")

    with tile.TileContext(nc) as tc, ExitStack() as ctx:
        persist = ctx.enter_context(tc.tile_pool(name="persist", bufs=1))
        acts = ctx.enter_context(tc.tile_pool(name="acts", bufs=3))
        temps = ctx.enter_context(tc.tile_pool(name="temps", bufs=3))
        psum = ctx.enter_context(tc.tile_pool(name="psum", bufs=1, space="PSUM"))
        psum1 = ctx.enter_context(tc.tile_pool(name="psum1", bufs=1, space="PSUM"))

        wh2_sb = persist.tile([2 * H, 4, 2 * H], DT, tag="wh2")
        nc.sync.dma_start(wh2_sb[:], wh2[:])
        wx2_sb = persist.tile([KX, 4, 2 * H], DT, tag="wx2")
        nc.sync.dma_start(wx2_sb[:], wx2[:])
        wfc2_sb = persist.tile([2 * H, 2 * O], DT, tag="wfc2")
        nc.sync.dma_start(wfc2_sb[:], wfc2[:])

        # Persistent state: cell state and the double-buffered hidden state.
        c_st = persist.tile([2 * H, NG], DT, tag="c")
        nc.vector.memset(c_st[:], 0.0)
        hbuf = []
        for j in range(2):
            hb = persist.tile([2 * H, NG], DT, tag=f"h{j}")
            nc.vector.memset(hb[:], 0.0)
            hbuf.append(hb)
        # Double-buffered [1; x] tiles (ones rows come in with the DMA).
        oxb = []
        for j in range(2):
            ox_t = persist.tile([KX, NG], DT, tag=f"ox{j}")
            oxb.append(ox_t)

        for t in range(T):
            hc = hbuf[t % 2]
            hn = hbuf[(t + 1) % 2]
            ox = oxb[t % 2]

            nc.sync.dma_start(ox[:], xT2[t])

            # PSUM: `start=True` clears the accumulate (has_written) bits of
            # its whole BANK, so each gate chunk gets a private 2 KB bank:
            # two 2-bank tiles with the second region offset by 512 fp32.
            BK = 512  # fp32 elements per PSUM bank (per partition)
            psIF = psum.tile([2 * H, 2 * BK], F32, tag="psIF")  # i @0, f @512
            psGO = psum.tile([2 * H, 2 * BK], F32, tag="psGO")  # g @0, o @512
            regions = {
                0: psIF[:, 0:NG],            # i
                1: psIF[:, BK : BK + NG],    # f
                2: psGO[:, 0:NG],            # g
                3: psGO[:, BK : BK + NG],    # o
            }

            # x/bias parts: one independent accumulation group per bank; all
            # pre-run in the PE's idle window while the h-matmuls wait for h.
            for ch in CHUNKS:
                nc.tensor.matmul(
                    regions[ch], wx2_sb[:, ch, :], ox[:], start=True, stop=False
                )
            # recurrent parts: the 4-matmul wave gating the step.
            for ch in CHUNKS:
                if STAT_M == 128:
                    nc.tensor.matmul(
                        regions[ch], wh2_sb[:, ch, :], hc[:], start=False, stop=True
                    )
                else:
                    for g in range(2):
                        nc.tensor.matmul(
                            regions[ch][g * H : (g + 1) * H, :],
                            wh2_sb[:, ch, g * H : (g + 1) * H],
                            hc[:],
                            start=False,
                            stop=True,
                        )

            tg = acts.tile([2 * H, NG], DT, tag="tg")
            nc.scalar.activation(tg[:], regions[2], AF.Tanh)
            sif = acts.tile([2 * H, 2 * NG], DT, tag="sif")
            nc.scalar.activation(
                sif[:],
                psIF[:].rearrange("p (b n) -> p b n", b=2)[:, :, 0:NG],
                AF.Sigmoid,
            )
            so = acts.tile([2 * H, NG], DT, tag="so")
            nc.scalar.activation(so[:], regions[3], AF.Sigmoid)

            si = sif[:, 0:NG]
            sf = sif[:, NG : 2 * NG]

            u = temps.tile([2 * H, NG], DT, tag="u")
            nc.vector.tensor_mul(u[:], si, tg[:])
            w = temps.tile([2 * H, NG], DT, tag="w")
            nc.vector.tensor_mul(w[:], sf, c_st[:])
            nc.vector.tensor_add(c_st[:], u[:], w[:])

            tcs = acts.tile([2 * H, NG], DT, tag="tc")
            nc.scalar.activation(tcs[:], c_st[:], AF.Tanh)

            nc.vector.tensor_mul(hn[:], so[:], tcs[:])

        # Final FC: one matmul, both groups ([O g0 | O g1] output rows).
        hf = hbuf[T % 2]
        fc_ps = psum1.tile([2 * O, NG], F32, tag="fc")
        nc.tensor.matmul(fc_ps[:], wfc2_sb[:], hf[:], start=True, stop=True)
        fc_sb = temps.tile([2 * O, NG], F32, tag="fcsb")
        nc.vector.tensor_copy(fc_sb[:], fc_ps[:])
        nc.sync.dma_start(out[:], fc_sb[:])

    nc.compile()
    return nc


def prep_weights(W_ih, W_hh, b_ih, b_hh, W_fc, b_fc):
    bsum = (b_ih + b_hh).astype(np.float32)
    wh2 = np.zeros((2 * H, 4, 2 * H), np.float32)
    wx2 = np.zeros((KX, 4, 2 * H), np.float32)
    for ch in range(4):
        r = slice(ch * H, (ch + 1) * H)
        wh2[0:H, ch, 0:H] = W_hh[r].T
        wh2[H:, ch, H:] = W_hh[r].T
        wx2[0, ch, 0:H] = bsum[r]
        wx2[1 : 1 + I, ch, 0:H] = W_ih[r].T
        wx2[1 + I, ch, H:] = bsum[r]
        wx2[2 + I :, ch, H:] = W_ih[r].T
    wfc2 = np.zeros((2 * H, 2 * O), np.float32)
    wfc2[0:H, 0:O] = W_fc.T
    wfc2[H:, O:] = W_fc.T
    return wh2, wx2, wfc2


def make_in_maps(x, W_ih, W_hh, b_ih, b_hh, W_fc, b_fc, T=T_FULL, use_bf16=None):
    import ml_dtypes

    if use_bf16 is None:
        use_bf16 = USE_BF16
    npdt = ml_dtypes.bfloat16 if use_bf16 else np.float32
    wh2, wx2, wfc2 = prep_weights(W_ih, W_hh, b_ih, b_hh, W_fc, b_fc)
    wh2, wx2, wfc2 = (a.astype(npdt) for a in (wh2, wx2, wfc2))
    in_maps = []
    for core in range(NCORES):
        xc = x[core * BLOC : (core + 1) * BLOC, :T, :]  # [BLOC, T, I]
        # xT2[t] = [1; x_g0^T; 1; x_g1^T]  -> [T, 10, NG]
        xT = np.ascontiguousarray(xc.transpose(1, 2, 0))  # [T, I, BLOC]
        xT2 = np.empty((T, KX, NG), np.float32)
        xT2[:, 0, :] = 1.0
        xT2[:, 1 : 1 + I, :] = xT[:, :, 0:NG]
        xT2[:, 1 + I, :] = 1.0
        xT2[:, 2 + I :, :] = xT[:, :, NG : 2 * NG]
        in_maps.append(
            {"xT2": xT2.astype(npdt), "wh2": wh2, "wx2": wx2, "wfc2": wfc2}
        )
    return in_maps


_CACHED_NC = None


def kernel(x, W_ih, W_hh, b_ih, b_hh, W_fc, b_fc):
    global _CACHED_NC
    x = np.asarray(x, np.float32)
    args = [np.asarray(a, np.float32) for a in (W_ih, W_hh, b_ih, b_hh, W_fc, b_fc)]
    if _CACHED_NC is None:
        _CACHED_NC = build_nc()
    nc = _CACHED_NC
    in_maps = make_in_maps(x, *args)
    res = run_bass_kernel_spmd(nc, in_maps, core_ids=list(range(NCORES)))
    b_fc = args[5]
    full = np.empty((1, B, O), np.float32)
    for core in range(NCORES):
        oc = res.results[core]["out"]  # [2*O, NG]
        for g in range(2):
            lo = core * BLOC + g * NG
            full[0, lo : lo + NG, :] = oc[g * O : (g + 1) * O].T + b_fc
    return full


# revision 16
# speedup vs baseline: 2.0634x; 1.0027x over previous
"""Trainium2 Bass kernel for a 1-layer LSTM (B=2048, T=512, I=4, H=64) + FC (O=4).

Sharding: data-parallel over batch across 8 NeuronCores (256 examples/core);
the tiny LSTM/FC weights are replicated.

On-core layout: SBUF partitions carry hidden/gate rows, the free dimension
carries batch.  The 256 local examples form two groups of 128; the groups are
stacked in the partition dimension (group 0 -> rows 0-63, group 1 -> rows
64-127) so ScalarE/VectorE instructions run with all 128 lanes busy.

The recurrent state is a single tile hbuf[128, 128] (both groups' h stacked).
Per step the gate pre-activations are built by PSUM accumulation of two
matmuls per gate chunk:
  mm_x (start=True):  stat Wx2[10, 128]  x  ox2[10, 128]   (bias+x part)
      ox2 rows: [1; x_g0 (4); 1; x_g1 (4)]; Wx2 block-maps each group's
      rows onto its 64 output columns.  These run OFF the critical path
      (the x DMA lands ~2 steps ahead).
  mm_h (stop=True):   stat Wh2[128, 128] x  hbuf[128, 128]  (recurrent part)
      Wh2 = blockdiag(W_hh_chunk^T, W_hh_chunk^T) so ONE matmul covers both
      groups; only 4 h-matmuls gate the step.
Then on ScalarE: sigmoid(i|f) (one strided-AP ACT across the two banks;
gates the c-update), tanh(g), sigmoid(o) (only needed after tanh(c), runs
in the ScalarE shadow), tanh(c); on VectorE: w = sf*c, u = si*tg,
c = u + w, and ONE h-update h = so * tanh(c) -> hbuf (both groups at once).

Measured: ~2444 ns/step, 1.245 ms total (24.6% faster than the 1.651 ms
baseline); rel err 4.54e-3 (bf16; gate < 2e-2).
"""

from contextlib import ExitStack

import numpy as np

import concourse.bass as bass
import concourse.tile as tile
from concourse import bacc, mybir
from concourse.bass_utils import run_bass_kernel_spmd

F32 = mybir.dt.float32
BF16 = mybir.dt.bfloat16
AF = mybir.ActivationFunctionType

H, I, O = 64, 4, 4
B, T_FULL = 2048, 512
NCORES = 8
BLOC = B // NCORES          # 256 examples per core
NG = 128                    # batch per group (2 groups per core)
KX = 2 * (1 + I)            # 10 rows of ox2: [1; x_g0; 1; x_g1]

# bf16 compute (matmuls, activations, cell state) keeps max rel err ~4e-3
USE_BF16 = True

# PE issue order of the gate chunks (ids: 0=i, 1=f, 2=g, 3=o): f and i
# first — sigmoid(i|f) gates the c-update and issues right after the 2nd
# h-matmul; tanh(g) is ScalarE-shadow-bound behind the sigmoid anyway, so
# g third costs nothing; o (only needed after tanh(c)) last.
CHUNKS = (1, 0, 2, 3)

# Stationary width for the recurrent matmuls: 128 = one block-diagonal
# matmul per chunk (both groups); 64 = two per chunk (one per group,
# cheaper LDWEIGHTS per matmul if the wide load bounds the wave rate).
STAT_M = 128


def build_nc(T=T_FULL, use_bf16=None):
    if use_bf16 is None:
        use_bf16 = USE_BF16
    DT = BF16 if use_bf16 else F32
    nc = bacc.Bacc(
        "TRN2",
        target_bir_lowering=False,
        debug=False,
        enable_asserts=False,
        num_devices=NCORES,
    )

    xT2 = nc.dram_tensor("xT2", [T, KX, NG], DT, kind="ExternalInput")
    wh2 = nc.dram_tensor("wh2", [2 * H, 4, 2 * H], DT, kind="ExternalInput")
    wx2 = nc.dram_tensor("wx2", [KX, 4, 2 * H], DT, kind="ExternalInput")
    wfc2 = nc.dram_tensor("wfc2", [2 * H, 2 * O], DT, kind="ExternalInput")
    out = nc.dram_tensor("out", [2 * O, NG], F32, kind="ExternalOutput")

    with tile.TileContext(nc) as tc, ExitStack() as ctx:
        persist = ctx.enter_context(tc.tile_pool(name="persist", bufs=1))
        acts = ctx.enter_context(tc.tile_pool(name="acts", bufs=3))
        temps = ctx.enter_context(tc.tile_pool(name="temps", bufs=3))
        psum = ctx.enter_context(tc.tile_pool(name="psum", bufs=1, space="PSUM"))
        psum1 = ctx.enter_context(tc.tile_pool(name="psum1", bufs=1, space="PSUM"))

        wh2_sb = persist.tile([2 * H, 4, 2 * H], DT, tag="wh2")
        nc.sync.dma_start(wh2_sb[:], wh2[:])
        wx2_sb = persist.tile([KX, 4, 2 * H], DT, tag="wx2")
        nc.sync.dma_start(wx2_sb[:], wx2[:])
        wfc2_sb = persist.tile([2 * H, 2 * O], DT, tag="wfc2")
        nc.sync.dma_start(wfc2_sb[:], wfc2[:])

        # Persistent state: cell state and the double-buffered hidden state.
        c_st = persist.tile([2 * H, NG], DT, tag="c")
        nc.vector.memset(c_st[:], 0.0)
        hbuf = []
        for j in range(2):
            hb = persist.tile([2 * H, NG], DT, tag=f"h{j}")
            nc.vector.memset(hb[:], 0.0)
            hbuf.append(hb)
        # Double-buffered [1; x] tiles (ones rows come in with the DMA).
        oxb = []
        for j in range(2):
            ox_t = persist.tile([KX, NG], DT, tag=f"ox{j}")
            oxb.append(ox_t)

        for t in range(T):
            hc = hbuf[t % 2]
            hn = hbuf[(t + 1) % 2]
            ox = oxb[t % 2]

            nc.sync.dma_start(ox[:], xT2[t])

            # PSUM: `start=True` clears the accumulate (has_written) bits of
            # its whole BANK, so each gate chunk gets a private 2 KB bank:
            # two 2-bank tiles with the second region offset by 512 fp32.
            BK = 512  # fp32 elements per PSUM bank (per partition)
            psIF = psum.tile([2 * H, 2 * BK], F32, tag="psIF")  # i @0, f @512
            psGO = psum.tile([2 * H, 2 * BK], F32, tag="psGO")  # g @0, o @512
            regions = {
                0: psIF[:, 0:NG],            # i
                1: psIF[:, BK : BK + NG],    # f
                2: psGO[:, 0:NG],            # g
                3: psGO[:, BK : BK + NG],    # o
            }

            # x/bias parts: one independent accumulation group per bank; all
            # pre-run in the PE's idle window while the h-matmuls wait for h.
            for ch in CHUNKS:
                nc.tensor.matmul(
                    regions[ch], wx2_sb[:, ch, :], ox[:], start=True, stop=False
                )
            # recurrent parts: the 4-matmul wave gating the step.
            for ch in CHUNKS:
                if STAT_M == 128:
                    nc.tensor.matmul(
                        regions[ch], wh2_sb[:, ch, :], hc[:], start=False, stop=True
                    )
                else:
                    for g in range(2):
                        nc.tensor.matmul(
                            regions[ch][g * H : (g + 1) * H, :],
                            wh2_sb[:, ch, g * H : (g + 1) * H],
                            hc[:],
                            start=False,
                            stop=True,
                        )

            tg = acts.tile([2 * H, NG], DT, tag="tg")
            nc.scalar.activation(tg[:], regions[2], AF.Tanh)
            sif = acts.tile([2 * H, 2 * NG], DT, tag="sif")
            nc.scalar.activation(
                sif[:],
                psIF[:].rearrange("p (b n) -> p b n", b=2)[:, :, 0:NG],
                AF.Sigmoid,
            )
            so = acts.tile([2 * H, NG], DT, tag="so")
            nc.scalar.activation(so[:], regions[3], AF.Sigmoid)

            si = sif[:, 0:NG]
            sf = sif[:, NG : 2 * NG]

            u = temps.tile([2 * H, NG], DT, tag="u")
            nc.vector.tensor_mul(u[:], si, tg[:])
            w = temps.tile([2 * H, NG], DT, tag="w")
            nc.vector.tensor_mul(w[:], sf, c_st[:])
            nc.vector.tensor_add(c_st[:], u[:], w[:])

            tcs = acts.tile([2 * H, NG], DT, tag="tc")
            nc.scalar.activation(tcs[:], c_st[:], AF.Tanh)

            nc.vector.tensor_mul(hn[:], so[:], tcs[:])

        # Final FC: one matmul, both groups ([O g0 | O g1] output rows).
        hf = hbuf[T % 2]
        fc_ps = psum1.tile([2 * O, NG], F32, tag="fc")
        nc.tensor.matmul(fc_ps[:], wfc2_sb[:], hf[:], start=True, stop=True)
        fc_sb = temps.tile([2 * O, NG], F32, tag="fcsb")
        nc.vector.tensor_copy(fc_sb[:], fc_ps[:])
        nc.sync.dma_start(out[:], fc_sb[:])

    nc.compile()
    return nc


def prep_weights(W_ih, W_hh, b_ih, b_hh, W_fc, b_fc):
    bsum = (b_ih + b_hh).astype(np.float32)
    wh2 = np.zeros((2 * H, 4, 2 * H), np.float32)
    wx2 = np.zeros((KX, 4, 2 * H), np.float32)
    for ch in range(4):
        r = slice(ch * H, (ch + 1) * H)
        wh2[0:H, ch, 0:H] = W_hh[r].T
        wh2[H:, ch, H:] = W_hh[r].T
        wx2[0, ch, 0:H] = bsum[r]
        wx2[1 : 1 + I, ch, 0:H] = W_ih[r].T
        wx2[1 + I, ch, H:] = bsum[r]
        wx2[2 + I :, ch, H:] = W_ih[r].T
    wfc2 = np.zeros((2 * H, 2 * O), np.float32)
    wfc2[0:H, 0:O] = W_fc.T
    wfc2[H:, O:] = W_fc.T
    return wh2, wx2, wfc2


def make_in_maps(x, W_ih, W_hh, b_ih, b_hh, W_fc, b_fc, T=T_FULL, use_bf16=None):
    import ml_dtypes

    if use_bf16 is None:
        use_bf16 = USE_BF16
    npdt = ml_dtypes.bfloat16 if use_bf16 else np.float32
    wh2, wx2, wfc2 = prep_weights(W_ih, W_hh, b_ih, b_hh, W_fc, b_fc)
    wh2, wx2, wfc2 = (a.astype(npdt) for a in (wh2, wx2, wfc2))
    in_maps = []
    for core in range(NCORES):
        xc = x[core * BLOC : (core + 1) * BLOC, :T, :]  # [BLOC, T, I]
        # xT2[t] = [1; x_g0^T; 1; x_g1^T]  -> [T, 10, NG]
        xT = np.ascontiguousarray(xc.transpose(1, 2, 0))  # [T, I, BLOC]
        xT2 = np.empty((T, KX, NG), np.float32)
        xT2[:, 0, :] = 1.0
        xT2[:, 1 : 1 + I, :] = xT[:, :, 0:NG]
        xT2[:, 1 + I, :] = 1.0
        xT2[:, 2 + I :, :] = xT[:, :, NG : 2 * NG]
        in_maps.append(
            {"xT2": xT2.astype(npdt), "wh2": wh2, "wx2": wx2, "wfc2": wfc2}
        )
    return in_maps


_CACHED_NC = None


def kernel(x, W_ih, W_hh, b_ih, b_hh, W_fc, b_fc):
    global _CACHED_NC
    x = np.asarray(x, np.float32)
    args = [np.asarray(a, np.float32) for a in (W_ih, W_hh, b_ih, b_hh, W_fc, b_fc)]
    if _CACHED_NC is None:
        _CACHED_NC = build_nc()
    nc = _CACHED_NC
    in_maps = make_in_maps(x, *args)
    res = run_bass_kernel_spmd(nc, in_maps, core_ids=list(range(NCORES)))
    b_fc = args[5]
    full = np.empty((1, B, O), np.float32)
    for core in range(NCORES):
        oc = res.results[core]["out"]  # [2*O, NG]
        for g in range(2):
            lo = core * BLOC + g * NG
            full[0, lo : lo + NG, :] = oc[g * O : (g + 1) * O].T + b_fc
    return full
